# revision 1
# baseline (speedup 1.0000x reference)
"""Trainium2 Bass kernel for AnchorMambaPoolingBlockGated.

Reference computation (per batch element b, channel-first x of shape (D, L)):
    xb = x.reshape(D, N, 2)                    # stride-2 blocks
    mu = xb.mean(-1); mx = xb.max(-1)          # (D, N)
    g  = sigmoid(W @ [mu; mx] + b)             # 1x1 conv over channels
    anchors = g*mx + (1-g)*mu
    out[3k]   = anchors[:, k]
    out[3k+1] = x[:, 2k]
    out[3k+2] = x[:, 2k+1]                     # out is (3N, D)  (transposed!)

Algebra used on device (verified against the reference in numpy):
    su = e + o            (e = even tokens, o = odd tokens)
    d2 = |e - o|
    z  = W1 @ su + W2 @ d2 + b      with  W1 = 0.5*(Wmu + Wmx), W2 = 0.5*Wmx
    g  = sigmoid(z)
    anchors = 0.5*(su + g*d2)

Sharding: data-parallel over batch, core i <- batch element i (B == 8 == n_cores).
No cross-core communication.  Gate weights replicated (pre-folded on host).

Layout strategy: all pooling/gate/blend work happens in (channel-partition,
token-free) layout; the final (L, D) transposed+interleaved output is produced
with PE transposes (matmul transpose mode, 128x128 tiles) assembled into
(token-partition, 512-channel) SBUF tiles which DMA out with a strided DRAM
access pattern that realizes the anchor/token interleave for free.
"""

import os
import numpy as np

import concourse.bass as bass
import concourse.tile as tile
from concourse import bacc, mybir
from concourse.alu_op_type import AluOpType
from concourse.bass_utils import run_bass_kernel_spmd

B, D, L = 8, 512, 8192
S = 2
N = L // S                # 4096 pooled blocks
LC = N * (S + 1)          # 12288 output rows per batch
P = 128
DC = D // P               # 4 channel chunks
KC = 2 * D // P           # 8 contraction chunks
NCORES = 8

CHUNK_L = 1024            # tokens processed per pipeline chunk
CHUNK_N = CHUNK_L // S    # 512 blocks per chunk

MODE = os.environ.get("KERNEL_MODE", "f16")   # "f32" (exact) or "f16"

_cache = {}


def _build(mode: str, l_total: int):
    """Build the per-core Bass program.  l_total lets tests compile a 1-chunk
    mini kernel; the real kernel uses l_total == L."""
    n_chunks = l_total // CHUNK_L
    n_total = l_total // S

    f32 = mybir.dt.float32
    f32r = mybir.dt.float32r
    f16 = mybir.dt.float16
    cdt = f32 if mode == "f32" else f16     # on-chip compute dtype
    mdt = f32r if mode == "f32" else f16    # matmul-operand dtype

    nc = bacc.Bacc("TRN2", target_bir_lowering=False, debug=False,
                   num_devices=NCORES)

    x_ext = nc.declare_dram_parameter("x", [D, l_total], f32, isOutput=False)
    wt_ext = nc.declare_dram_parameter("wt", [2 * D, D], mdt, isOutput=False)
    bias_ext = nc.declare_dram_parameter("bias", [D, 1], f32, isOutput=False)
    id_ext = nc.declare_dram_parameter("ident", [P, P], cdt, isOutput=False)
    out_ext = nc.declare_dram_parameter("out", [n_total, 3, D], f32,
                                        isOutput=True)

    with tile.TileContext(nc) as tc:
        with (
            tc.tile_pool(name="consts", bufs=1) as p_const,
            tc.tile_pool(name="xin", bufs=4) as p_x,
            tc.tile_pool(name="long", bufs=8) as p_long,
            tc.tile_pool(name="short", bufs=4) as p_short,
            tc.tile_pool(name="outx", bufs=4) as p_ox,
            tc.tile_pool(name="outa", bufs=2) as p_oa,
            tc.tile_pool(name="psz", bufs=4, space="PSUM") as p_psz,
            tc.tile_pool(name="pst", bufs=4, space="PSUM") as p_pst,
        ):
            # --- constants ---------------------------------------------------
            wt_sb = p_const.tile([P, KC, D], mdt)               # (128, 8, 512)
            nc.sync.dma_start(wt_sb[:],
                              wt_ext.rearrange("(k p) d -> p k d", p=P))
            ident = p_const.tile([P, P], cdt)
            nc.sync.dma_start(ident[:], id_ext[:])
            bias_sb = p_const.tile([P, DC, 1], f32)             # (128, 4, 1)
            nc.sync.dma_start(bias_sb[:],
                              bias_ext.rearrange("(m p) o -> p m o", p=P))

            def load_chunk(ci):
                l0 = ci * CHUNK_L
                xt = p_x.tile([P, DC, CHUNK_L], cdt, tag="xt")
                src = x_ext.rearrange("(c p) l -> p c l", p=P)[:, :, l0:l0 + CHUNK_L]
                if cdt is f32:
                    nc.sync.dma_start(xt[:], src)
                else:
                    nc.gpsimd.dma_start(xt[:], src)   # SWDGE casts f32->f16
                return xt

            for ci in range(n_chunks):
                l0 = ci * CHUNK_L           # first token of chunk
                n0 = ci * CHUNK_N           # first block of chunk
                xt = load_chunk(ci)

                # --- pooling: su = e+o, d2 = |e-o| ---------------------------
                su, d2 = [], []
                for dc in range(DC):
                    e = xt[:, dc, 0::2]
                    o = xt[:, dc, 1::2]
                    s = p_long.tile([P, CHUNK_N], mdt, tag="su")
                    nc.vector.tensor_tensor(s[:], e, o, AluOpType.add)
                    su.append(s)
                    t_ = p_short.tile([P, CHUNK_N], cdt, tag="tdiff")
                    nc.vector.tensor_tensor(t_[:], e, o, AluOpType.subtract)
                    d = p_long.tile([P, CHUNK_N], mdt, tag="d2")
                    # |t| on the scalar engine to keep DVE free
                    nc.scalar.activation(d[:], t_[:],
                                         mybir.ActivationFunctionType.Abs)
                    d2.append(d)

                # --- x tokens: transpose + copy + interleaved DMA out --------
                for jm in range(CHUNK_L // P // 4):
                    xtt = p_ox.tile([P, 4, D], f32, tag="xtile")
                    for h in range(4):
                        j = jm * 4 + h
                        ps = p_pst.tile([P, D], cdt, tag="pst")
                        for dc in range(DC):
                            nc.tensor.transpose(
                                ps[:, dc * P:(dc + 1) * P],
                                xt[:, dc, j * P:(j + 1) * P],
                                ident[:])
                        if j % 2 == 0:
                            nc.scalar.copy(xtt[:, h, :], ps[:])
                        else:
                            nc.vector.tensor_copy(xtt[:, h, :], ps[:])
                    bk = (l0 + jm * 4 * P) // S      # 256 output pair-rows
                    dv1 = out_ext[bk:bk + 256, 1, :].rearrange(
                        "(h a) d -> a h d", h=4)
                    dv2 = out_ext[bk:bk + 256, 2, :].rearrange(
                        "(h a) d -> a h d", h=4)
                    nc.sync.dma_start(dv1, xtt[0::2])
                    nc.sync.dma_start(dv2, xtt[1::2])

                # --- gate matmul + sigmoid + blend ---------------------------
                anch = []
                for md in range(DC):
                    ps = p_psz.tile([P, CHUNK_N], f32, tag="psz")
                    for kc in range(KC):
                        lhsT = wt_sb[:, kc, md * P:(md + 1) * P]
                        rhs = (su[kc][:] if kc < DC else d2[kc - DC][:])
                        nc.tensor.matmul(ps[:], lhsT, rhs,
                                         start=(kc == 0),
                                         stop=(kc == KC - 1))
                    g = p_short.tile([P, CHUNK_N], cdt, tag="g")
                    nc.scalar.activation(g[:], ps[:],
                                         mybir.ActivationFunctionType.Sigmoid,
                                         bias=bias_sb[:, md, :])
                    h = p_short.tile([P, CHUNK_N], cdt, tag="h")
                    nc.vector.tensor_tensor(h[:], g[:], d2[md][:],
                                            AluOpType.mult)
                    ar = p_long.tile([P, CHUNK_N], cdt, tag="ar")
                    nc.vector.tensor_tensor(ar[:], su[md][:], h[:],
                                            AluOpType.add)
                    anch.append(ar)

                # --- anchors: transpose + copy + merged interleaved DMA out --
                at = p_oa.tile([P, CHUNK_N // P, D], f32, tag="atile")
                for j2 in range(CHUNK_N // P):
                    ps = p_pst.tile([P, D], cdt, tag="pst")
                    for md in range(DC):
                        nc.tensor.transpose(
                            ps[:, md * P:(md + 1) * P],
                            anch[md][:, j2 * P:(j2 + 1) * P],
                            ident[:])
                    nc.scalar.mul(at[:, j2, :], ps[:], 0.5)
                aview = out_ext[n0:n0 + CHUNK_N, 0, :]
                aview = aview.rearrange("(h p) d -> p h d", h=CHUNK_N // P)
                nc.sync.dma_start(aview, at[:])

    nc.compile()
    return nc


def _get_nc(mode=MODE, l_total=L):
    key = (mode, l_total)
    if key not in _cache:
        _cache[key] = _build(mode, l_total)
    return _cache[key]


def _prep_weights(gate_w: np.ndarray, mode: str):
    gw = np.asarray(gate_w, dtype=np.float32)
    w_mu, w_mx = gw[:, :D], gw[:, D:]
    w1 = 0.5 * (w_mu + w_mx)
    w2 = 0.5 * w_mx
    wt = np.concatenate([w1.T, w2.T], axis=0)        # (2D, D), wt[c, d]
    dt = np.float32 if mode == "f32" else np.float16
    return np.ascontiguousarray(wt.astype(dt))


LAST_RESULTS = None


def kernel(x, gate_w, gate_b, mask):
    global LAST_RESULTS
    mode = MODE
    nc = _get_nc(mode, L)

    x = np.asarray(x, dtype=np.float32)
    wt = _prep_weights(gate_w, mode)
    bias = np.ascontiguousarray(np.asarray(gate_b, np.float32).reshape(D, 1))
    dt = np.float32 if mode == "f32" else np.float16
    ident = np.eye(P, dtype=dt)

    in_maps = [
        {"x": np.ascontiguousarray(x[b]), "wt": wt, "bias": bias,
         "ident": ident}
        for b in range(NCORES)
    ]
    res = run_bass_kernel_spmd(nc, in_maps, core_ids=list(range(NCORES)))
    LAST_RESULTS = res
    out = np.stack([res.results[i]["out"].reshape(LC, D)
                    for i in range(NCORES)])
    return out.astype(np.float32, copy=False)

